# revision 1
# baseline (speedup 1.0000x reference)
# Trainium2 Bass kernel for nn_CombinedLoss (CE + proto-assignment + SupCon + proto-orthogonality)
#
# Strategy (8 NeuronCores, data-parallel over batch):
#   - Each core gets a 1024-row shard of logits/embeddings/labels.
#   - Segment sums (per-class prototype sums, counts, z-sums S_c, z-sumsq ssq_c) are
#     computed with one-hot matmuls on the shard and AllReduced across cores.
#   - Normalized embeddings z are transposed per-shard on the TensorEngine and
#     AllGathered; each core loads the gathered blocks ROTATED so its own block sits
#     at columns [0,1024) -> the sim-matrix diagonal lands at a compile-time position.
#   - SupCon: per-row only logsumexp(sim) is needed.  The positive-pair term
#     collapses to class space:  sum_{i in c} sum_{j in pos(i)} sim_ij
#       = (||S_c||^2 - ssq_c)/tau,   pos_count_i = cnt_c - 1.
#     lse is segment-summed per class with one-hot matmuls and AllReduced (tiny).
#   - All big matmuls run as float32r (FP22, 1 cycle/row).
#
# Output matches reference: tuple (total, loss1, loss2, loss3, loss4) of fp32 scalars.

import numpy as np

B = 8192
C = 512  # NUM_CLASSES
D = 256
NCORES = 8
SH = B // NCORES  # 1024 rows per core
T = SH // 128  # 8 row-tiles per core
ALPHA = 0.5
BETA = 0.5
GAMMA = 0.5
INV_TAU = 10.0
EPS = 1e-8

_CACHE = {}


def _build():
    import concourse.bass as bass
    import concourse.mybir as mybir
    import concourse.tile as tile
    from concourse import bacc, bass_isa
    from concourse.masks import make_identity

    f32 = mybir.dt.float32
    f32r = mybir.dt.float32r
    i32 = mybir.dt.int32
    AX = mybir.AxisListType
    OP = mybir.AluOpType
    ACT = mybir.ActivationFunctionType

    nc = bacc.Bacc("TRN2", target_bir_lowering=False, debug=False, num_devices=NCORES)

    lg_in = nc.dram_tensor("logits", [SH, C], f32, kind="ExternalInput")
    em_in = nc.dram_tensor("emb", [SH, D], f32r, kind="ExternalInput")
    lab_in = nc.dram_tensor("labels_f", [128, T], f32, kind="ExternalInput")
    out_losses = nc.dram_tensor("partials", [128, 8], f32, kind="ExternalOutput")
    import os
    _dbg = os.environ.get("KERNEL_DEBUG", "") == "1"
    if _dbg:
        dbg_out = nc.dram_tensor("dbg", [128, 8 * T], f32, kind="ExternalOutput")

    with tile.TileContext(nc) as tc:
        with (
            tc.tile_pool(name="const", bufs=1) as constp,
            tc.tile_pool(name="persist", bufs=1) as pers,
            tc.tile_pool(name="scratch", bufs=3) as scr,
            tc.tile_pool(name="dram", bufs=1, space="DRAM") as dram,
        ):
            # ---------- constants ----------
            ident = constp.tile([128, 128], f32, name="ident")
            make_identity(nc, ident)
            ident_r = constp.tile([128, 128], f32r, name="ident_r")
            nc.vector.tensor_copy(ident_r, ident)
            ones_c = constp.tile([128, 1], f32, name="ones_c")
            nc.vector.memset(ones_c, 1.0)
            ones2 = constp.tile([128, 2], f32, name="ones2")
            nc.vector.memset(ones2, 1.0)
            onemI = constp.tile([128, 128], f32, name="onemI")
            nc.vector.memset(onemI, 1.0)
            nc.gpsimd.affine_select(
                out=onemI, in_=onemI, compare_op=OP.not_equal, fill=0.0,
                base=0, pattern=[[-1, 128]], channel_multiplier=1,
            )
            iota_i = constp.tile([128, C], i32, name="iota_i")
            nc.gpsimd.iota(iota_i, pattern=[[1, C]], base=0, channel_multiplier=0)
            iota_f = constp.tile([128, C], f32, name="iota_f")
            nc.vector.tensor_copy(iota_f, iota_i)

            lab = constp.tile([128, T], f32, name="lab")
            nc.sync.dma_start(lab, lab_in[:, :])

            # ---------- persistent tiles ----------
            e_ext = [pers.tile([128, D + 2], f32r, name=f"e_ext{t}") for t in range(T)]
            z_ext = [pers.tile([128, D + 2], f32r, name=f"z_ext{t}") for t in range(T)]
            O_t = [pers.tile([128, C], f32r, name=f"onehot{t}") for t in range(T)]
            ztf = [pers.tile([128, B], f32r, name=f"ztf{d}") for d in range(2)]
            zts = [pers.tile([128, SH], f32r, name=f"zts{d}") for d in range(2)]
            ssqs = pers.tile([128, T], f32, name="ssqs")
            ce_sums = pers.tile([128, T], f32, name="ce_sums")
            gls = pers.tile([128, T], f32, name="gls")
            rowsums = pers.tile([128, T], f32, name="rowsums")
            zden = pers.tile([128, T], f32, name="zden")
            finals = pers.tile([128, 8], f32, name="finals")

            # ---------- DRAM scratch ----------
            zt_local = dram.tile([D, SH], f32r, name="zt_local")
            zt_gath = dram.tile([NCORES, D, SH], f32r, name="zt_gath", addr_space="Shared")
            seg_in = dram.tile([128, 4, 2, 257], f32, name="seg_in")
            seg_out = dram.tile([128, 4, 2, 257], f32, name="seg_out", addr_space="Shared")

            # ================= Phase A : shard-local prep =================
            # load embeddings; row sums of squares
            for t in range(T):
                nc.sync.dma_start(e_ext[t][:, :D], em_in[t * 128:(t + 1) * 128, :])
                nc.vector.tensor_copy(e_ext[t][:, D:D + 2], ones2)
            for t in range(T):
                sq = scr.tile([128, D], f32, name="sq", tag="sq")
                nc.vector.scalar_tensor_tensor(
                    out=sq, in0=e_ext[t][:, :D], scalar=1.0, in1=e_ext[t][:, :D],
                    op0=OP.mult, op1=OP.mult, accum_out=ssqs[:, t:t + 1],
                )
            # norms: sqrt + one Newton step, then zden = 1/(norm + eps)
            n0 = constp.tile([128, T], f32, name="n0")
            nc.scalar.activation(n0, ssqs, ACT.Sqrt)
            n0m = constp.tile([128, T], f32, name="n0m")
            nc.vector.tensor_scalar(n0m, n0, 1e-20, None, OP.max)
            r0 = constp.tile([128, T], f32, name="r0")
            nc.vector.reciprocal(r0, n0m)
            t1 = constp.tile([128, T], f32, name="t1")
            nc.vector.tensor_tensor(t1, ssqs, r0, OP.mult)
            nc.vector.tensor_tensor(t1, t1, n0m, OP.add)
            nc.vector.tensor_scalar(t1, t1, 0.5, EPS, OP.mult, OP.add)
            nc.vector.reciprocal(zden, t1)

            # z tiles, one-hot tiles, zz column
            for t in range(T):
                nc.vector.tensor_scalar(
                    z_ext[t][:, :D], e_ext[t][:, :D], zden[:, t:t + 1], None, OP.mult
                )
                sq2 = scr.tile([128, D], f32, name="sq2", tag="sq")
                nc.vector.scalar_tensor_tensor(
                    out=sq2, in0=z_ext[t][:, :D], scalar=1.0, in1=z_ext[t][:, :D],
                    op0=OP.mult, op1=OP.mult, accum_out=z_ext[t][:, D:D + 1],
                )
                nc.vector.tensor_copy(z_ext[t][:, D + 1:D + 2], ones_c)
                nc.vector.tensor_scalar(O_t[t], iota_f, lab[:, t:t + 1], None, OP.is_equal)

            # transpose z -> zts (shard, [d, i] layout), then DMA out + AllGather
            with tc.tile_pool(name="trps", bufs=2, space="PSUM") as trps:
                for t in range(T):
                    for d in range(2):
                        ptr = trps.tile([128, 128], f32r, name="ptr", tag="ptr")
                        nc.tensor.transpose(ptr, z_ext[t][:, d * 128:(d + 1) * 128], ident_r)
                        nc.vector.tensor_copy(zts[d][:, t * 128:(t + 1) * 128], ptr)
            for d in range(2):
                nc.sync.dma_start(zt_local[d * 128:(d + 1) * 128, :], zts[d])
            nc.gpsimd.collective_compute(
                "AllGather", OP.bypass,
                replica_groups=[list(range(NCORES))],
                ins=[zt_local.opt()], outs=[zt_gath.opt()],
            )

            # CE pieces (ACT is on exp table now; sqrt was done above)
            for t in range(T):
                lgt = scr.tile([128, C], f32, name="lgt", tag="lgt")
                nc.sync.dma_start(lgt, lg_in[t * 128:(t + 1) * 128, :])
                esc = scr.tile([128, C], f32, name="esc", tag="esc")
                nc.scalar.activation(esc, lgt, ACT.Exp, accum_out=ce_sums[:, t:t + 1])
                gsc = scr.tile([128, C], f32, name="gsc", tag="gsc")
                nc.vector.scalar_tensor_tensor(
                    out=gsc, in0=O_t[t], scalar=1.0, in1=lgt,
                    op0=OP.mult, op1=OP.mult, accum_out=gls[:, t:t + 1],
                )

            # segment matmuls: accumulate over the 8 row tiles
            with tc.tile_pool(name="segps", bufs=1, space="PSUM") as segpsp:
                segps = [
                    segpsp.tile([128, 2, 512], f32, name=f"segps{cb}") for cb in range(4)
                ]
                for t in range(T):
                    for cb in range(4):
                        lhs = O_t[t][:, cb * 128:(cb + 1) * 128]
                        nc.tensor.matmul(
                            segps[cb][:, 0, :D + 2], lhs, e_ext[t][:, :],
                            start=(t == 0), stop=(t == T - 1),
                        )
                        nc.tensor.matmul(
                            segps[cb][:, 1, :D + 2], lhs, z_ext[t][:, :],
                            start=(t == 0), stop=(t == T - 1),
                        )
                # PSUM -> SBUF -> DRAM, AllReduce
                seg_sb = pers.tile([128, 4, 2, 257], f32, name="seg_sb")
                for cb in range(4):
                    for h in range(2):
                        nc.vector.tensor_copy(seg_sb[:, cb, h, :], segps[cb][:, h, :D + 1])
                nc.sync.dma_start(seg_in[:, :, :, :], seg_sb)
            nc.gpsimd.collective_compute(
                "AllReduce", OP.add,
                replica_groups=[list(range(NCORES))],
                ins=[seg_in.opt()], outs=[seg_out.opt()],
            )

            # load gathered zT with per-core rotation: block b <- (b + pid) % 8
            pid = nc.sync.partition_id()
            for d in range(2):
                nc.sync.dma_start(ztf[d][:, 0:SH], zt_local[d * 128:(d + 1) * 128, :])
            for blk in range(1, NCORES):
                src = (pid + blk) % NCORES
                for d in range(2):
                    nc.sync.dma_start(
                        ztf[d][:, blk * SH:(blk + 1) * SH],
                        zt_gath[bass.ds(src, 1), d * 128:(d + 1) * 128, :],
                    )

            # ================= Phase B : sim rows, exp, row-sums =================
            with tc.tile_pool(name="simps", bufs=2, space="PSUM") as simpsp:
                for r in range(T):
                    rs4 = scr.tile([128, 4], f32, name="rs4", tag="rs4")
                    for jc in range(4):
                        ps = simpsp.tile([128, 2048], f32, name="ps", tag="ps")
                        for d in range(2):
                            lhs = ztf[d][:, r * 128:(r + 1) * 128]
                            for jb in range(4):
                                nc.tensor.matmul(
                                    ps[:, jb * 512:(jb + 1) * 512],
                                    lhs,
                                    ztf[d][:, jc * 2048 + jb * 512: jc * 2048 + (jb + 1) * 512],
                                    start=(d == 0), stop=(d == 1),
                                )
                        if jc == 0:
                            # zero the diagonal block (own rows are at columns r*128..)
                            nc.vector.tensor_tensor(
                                ps[:, r * 128:(r + 1) * 128],
                                ps[:, r * 128:(r + 1) * 128], onemI, OP.mult,
                            )
                        ex = scr.tile([128, 2048], f32, name="ex", tag="ex")
                        nc.scalar.activation(
                            ex, ps, ACT.Exp, scale=INV_TAU, accum_out=rs4[:, jc:jc + 1]
                        )
                    rst = scr.tile([128, 1], f32, name="rst", tag="rst")
                    nc.vector.reduce_sum(rst, rs4, axis=AX.X)
                    # remove the exp(0)=1 the zeroed diagonal contributed
                    nc.vector.tensor_scalar(rowsums[:, r:r + 1], rst, -1.0, None, OP.add)

            # ================= Phase C : class-space finish =================
            lse = pers.tile([128, T], f32r, name="lse")
            nc.scalar.activation(lse, rowsums, ACT.Ln)
            lse_ce = pers.tile([128, T], f32, name="lse_ce")
            nc.scalar.activation(lse_ce, ce_sums, ACT.Ln)

            # loss1 partial: sum over shard of (lse_ce - gathered_logit)
            ced = scr.tile([128, T], f32, name="ced", tag="ced")
            nc.vector.tensor_tensor(ced, lse_ce, gls, OP.subtract)
            celoc = pers.tile([128, 1], f32, name="celoc")
            nc.vector.reduce_sum(celoc, ced, axis=AX.X)
            sseloc = pers.tile([128, 1], f32, name="sseloc")
            nc.vector.reduce_sum(sseloc, ssqs, axis=AX.X)

            # global segment sums (AllReduce #1 result)
            sseg = pers.tile([128, 4, 2, 257], f32, name="sseg")
            nc.sync.dma_start(sseg, seg_out[:, :, :, :])

            cnts = pers.tile([128, 4], f32, name="cnts")
            ssqc = pers.tile([128, 4], f32, name="ssqc")
            for cb in range(4):
                nc.vector.tensor_copy(cnts[:, cb:cb + 1], sseg[:, cb, 0, D:D + 1])
                nc.vector.tensor_copy(ssqc[:, cb:cb + 1], sseg[:, cb, 1, D:D + 1])

            cntm = pers.tile([128, 4], f32, name="cntm")
            nc.vector.tensor_scalar(cntm, cnts, 1.0, None, OP.max)
            rcnt = pers.tile([128, 4], f32, name="rcnt")
            nc.vector.reciprocal(rcnt, cntm)
            cm1 = pers.tile([128, 4], f32, name="cm1")
            nc.vector.tensor_scalar(cm1, cnts, -1.0, 1.0, OP.add, OP.max)
            rcm1 = pers.tile([128, 4], f32, name="rcm1")
            nc.vector.reciprocal(rcm1, cm1)
            v2 = pers.tile([128, 4], f32, name="v2")
            nc.vector.tensor_scalar(v2, cnts, 2.0, None, OP.is_ge)
            v1 = pers.tile([128, 4], f32, name="v1")
            nc.vector.tensor_scalar(v1, cnts, 0.5, None, OP.is_ge)

            # prototypes, ||p_c||^2, ||S_c||^2
            protos = [pers.tile([128, D], f32, name=f"protos{cb}") for cb in range(4)]
            pn2 = pers.tile([128, 4], f32, name="pn2")
            S2 = pers.tile([128, 4], f32, name="S2")
            for cb in range(4):
                nc.vector.tensor_scalar(
                    protos[cb], sseg[:, cb, 0, :D], rcnt[:, cb:cb + 1], None, OP.mult
                )
                psq = scr.tile([128, D], f32, name="psq", tag="sq")
                nc.vector.scalar_tensor_tensor(
                    out=psq, in0=protos[cb], scalar=1.0, in1=protos[cb],
                    op0=OP.mult, op1=OP.mult, accum_out=pn2[:, cb:cb + 1],
                )
                ssq2 = scr.tile([128, D], f32, name="ssq2", tag="sq")
                nc.vector.scalar_tensor_tensor(
                    out=ssq2, in0=sseg[:, cb, 1, :D], scalar=1.0, in1=sseg[:, cb, 1, :D],
                    op0=OP.mult, op1=OP.mult, accum_out=S2[:, cb:cb + 1],
                )

            # loss3 class terms (seg part, core-identical)
            t3 = pers.tile([128, 4], f32, name="t3")
            nc.vector.tensor_tensor(t3, S2, ssqc, OP.subtract)
            nc.vector.tensor_scalar(t3, t3, INV_TAU, None, OP.mult)
            nc.vector.tensor_tensor(t3, t3, rcm1, OP.mult)
            nc.vector.tensor_tensor(t3, t3, v2, OP.mult)
            nc.vector.reduce_sum(finals[:, 0:1], t3, axis=AX.X)
            nval = scr.tile([128, 4], f32, name="nval", tag="s4")
            nc.vector.tensor_tensor(nval, v2, cnts, OP.mult)
            nc.vector.reduce_sum(finals[:, 2:3], nval, axis=AX.X)

            # loss2: sum_c cnt*||p||^2
            cpn = scr.tile([128, 4], f32, name="cpn", tag="s4")
            nc.vector.tensor_tensor(cpn, cnts, pn2, OP.mult)
            nc.vector.reduce_sum(finals[:, 3:4], cpn, axis=AX.X)

            # loss4: normalized, masked prototypes and their Gram matrix
            pnorm = pers.tile([128, 4], f32, name="pnorm")
            nc.scalar.activation(pnorm, pn2, ACT.Sqrt)
            pnm = scr.tile([128, 4], f32, name="pnm", tag="s4b")
            nc.vector.tensor_scalar(pnm, pnorm, 1e-20, None, OP.max)
            pr0 = scr.tile([128, 4], f32, name="pr0", tag="s4c")
            nc.vector.reciprocal(pr0, pnm)
            pt1 = scr.tile([128, 4], f32, name="pt1", tag="s4d")
            nc.vector.tensor_tensor(pt1, pn2, pr0, OP.mult)
            nc.vector.tensor_tensor(pt1, pt1, pnm, OP.add)
            nc.vector.tensor_scalar(pt1, pt1, 0.5, EPS, OP.mult, OP.add)
            pden = pers.tile([128, 4], f32, name="pden")
            nc.vector.reciprocal(pden, pt1)
            nc.vector.tensor_tensor(pden, pden, v1, OP.mult)

            pnz = [pers.tile([128, D], f32r, name=f"pnz{cb}") for cb in range(4)]
            d2 = pers.tile([128, 4], f32, name="d2")
            for cb in range(4):
                nc.vector.tensor_scalar(
                    pnz[cb], protos[cb], pden[:, cb:cb + 1], None, OP.mult
                )
                dsq = scr.tile([128, D], f32, name="dsq", tag="sq")
                nc.vector.scalar_tensor_tensor(
                    out=dsq, in0=pnz[cb], scalar=1.0, in1=pnz[cb],
                    op0=OP.mult, op1=OP.mult, accum_out=d2[:, cb:cb + 1],
                )

            pnzT = [pers.tile([128, C], f32r, name=f"pnzT{d}") for d in range(2)]
            g2 = pers.tile([128, 4], f32, name="g2")
            with tc.tile_pool(name="gps", bufs=2, space="PSUM") as gpsp:
                for cb in range(4):
                    for d in range(2):
                        ptr2 = gpsp.tile([128, 128], f32r, name="ptr2", tag="ptr2")
                        nc.tensor.transpose(ptr2, pnz[cb][:, d * 128:(d + 1) * 128], ident_r)
                        nc.vector.tensor_copy(pnzT[d][:, cb * 128:(cb + 1) * 128], ptr2)
                for cb in range(4):
                    gp = gpsp.tile([128, C], f32, name="gp", tag="gp")
                    for d in range(2):
                        nc.tensor.matmul(
                            gp,
                            pnzT[d][:, cb * 128:(cb + 1) * 128],
                            pnzT[d][:, :],
                            start=(d == 0), stop=(d == 1),
                        )
                    gsq = scr.tile([128, C], f32, name="gsq", tag="gsq")
                    nc.scalar.activation(gsq, gp, ACT.Square, accum_out=g2[:, cb:cb + 1])
            d2sq = scr.tile([128, 4], f32, name="d2sq", tag="s4")
            nc.vector.tensor_tensor(d2sq, d2, d2, OP.mult)
            g2r = scr.tile([128, 1], f32, name="g2r", tag="rst")
            nc.vector.reduce_sum(g2r, g2, axis=AX.X)
            d2r = scr.tile([128, 1], f32, name="d2r", tag="rst")
            nc.vector.reduce_sum(d2r, d2sq, axis=AX.X)
            nc.vector.tensor_tensor(finals[:, 4:5], g2r, d2r, OP.subtract)
            nc.vector.reduce_sum(finals[:, 5:6], v1, axis=AX.X)

            # segment-sum of lse by class (per-core partial), v2-masked
            with tc.tile_pool(name="cps", bufs=1, space="PSUM") as cps:
                # one PSUM bank per class-block: matmul start=True clears the
                # whole bank, so accumulation groups must not share banks
                lseps = [cps.tile([128, 2], f32, name=f"lseps{cb}") for cb in range(4)]
                lsep = pers.tile([128, 2], f32r, name="lsep")
                nc.vector.tensor_copy(lsep[:, 1:2], ones_c)
                for t in range(T):
                    nc.vector.tensor_copy(lsep[:, 0:1], lse[:, t:t + 1])
                    for cb in range(4):
                        nc.tensor.matmul(
                            lseps[cb],
                            O_t[t][:, cb * 128:(cb + 1) * 128],
                            lsep,
                            start=(t == 0), stop=(t == T - 1),
                        )
                lsS = pers.tile([128, 4], f32, name="lsS")
                for cb in range(4):
                    nc.vector.tensor_copy(lsS[:, cb:cb + 1], lseps[cb][:, 0:1])
            nc.vector.tensor_tensor(lsS, lsS, v2, OP.mult)
            nc.vector.reduce_sum(finals[:, 1:2], lsS, axis=AX.X)

            nc.vector.tensor_copy(finals[:, 6:7], celoc)
            nc.vector.tensor_copy(finals[:, 7:8], sseloc)

            nc.sync.dma_start(out_losses[:, :], finals)
            if _dbg:
                dbg_sb = pers.tile([128, 8 * T], f32, name="dbg_sb")
                nc.vector.tensor_copy(dbg_sb[:, 0:T], rowsums)
                nc.vector.tensor_copy(dbg_sb[:, T:2 * T], lse)
                nc.vector.tensor_copy(dbg_sb[:, 2 * T:3 * T], ce_sums)
                nc.vector.memset(dbg_sb[:, 3 * T:8 * T], 0.0)
                nc.sync.dma_start(dbg_out[:, :], dbg_sb)

    nc.compile()
    return nc


def _get_nc():
    if "nc" not in _CACHE:
        _CACHE["nc"] = _build()
    return _CACHE["nc"]


def kernel(logits, embeddings, labels):
    from concourse import bass_utils

    nc = _get_nc()

    logits = np.ascontiguousarray(np.asarray(logits, dtype=np.float32))
    embeddings = np.ascontiguousarray(np.asarray(embeddings, dtype=np.float32))
    labels_np = np.asarray(labels)

    in_maps = []
    for c in range(NCORES):
        sl = slice(c * SH, (c + 1) * SH)
        lab_f = labels_np[sl].astype(np.float32).reshape(T, 128).T
        in_maps.append({
            "logits": logits[sl],
            "emb": embeddings[sl],
            "labels_f": np.ascontiguousarray(lab_f),
        })

    res = bass_utils.run_bass_kernel_spmd(nc, in_maps, core_ids=list(range(NCORES)))

    # finalize: partials cols = [t3a, t3b(lseS partial), nvalid, cnt*pn2, l4num,
    # npres, celoc(partial), sseloc(partial)]; per-partition class/row sums.
    p0 = res.results[0]["partials"].astype(np.float64)
    t3a = p0[:, 0].sum()
    nvalid = p0[:, 2].sum()
    cntpn2 = p0[:, 3].sum()
    l4num = p0[:, 4].sum()
    npres = p0[:, 5].sum()
    t3b = ce = sse = 0.0
    for c in range(NCORES):
        pc = res.results[c]["partials"].astype(np.float64)
        t3b += pc[:, 1].sum()
        ce += pc[:, 6].sum()
        sse += pc[:, 7].sum()

    l1 = ce / B
    l2 = (sse - cntpn2) / B
    l3 = -(t3a - t3b) / max(nvalid, 1.0)
    l4 = l4num / max(npres * npres - npres, 1.0)
    total = l1 + ALPHA * l2 + BETA * l3 + GAMMA * l4
    return tuple(np.float32(v) for v in (total, l1, l2, l3, l4))



# revision 9
# speedup vs baseline: 30.8077x; 30.8077x over previous
# Trainium2 Bass kernel for nn_CombinedLoss (CE + proto-assignment + SupCon + proto-orthogonality)
#
# Strategy (8 NeuronCores, data-parallel over batch, COLLECTIVE-FREE):
#   - Each core gets its own 1024-row shard of logits/labels/embeddings
#     (for CE, segment sums, and the sim-matrix rows) plus a replicated
#     copy of ALL embeddings (fp16) so it can build the full z^T [256,8192]
#     locally.  No AllGather / AllReduce: cross-core combination of the
#     small per-core partials (segment sums [512,512], row-sums [1024],
#     CE pieces) happens on the host, so the 8 cores never synchronize
#     with each other on device.  This removes the collective stalls that
#     dominated the previous version (each exec forced an 8-core
#     rendezvous; the dispatch skew was absorbed as on-device wait).
#   - z^T is built on-device: square-accumulate (Pool engine), a
#     Newton-refined rsqrt chain, scale (DVE), then SBUF->SBUF DMA
#     transposes (XBAR) -- no PE transposes, no PSUM traffic.
#   - Sim rows: out = z_ownT.T @ z_allT in fp16 (PE), exp+row-accumulate
#     on the Activation engine.  The diagonal is NOT masked on-device;
#     exp(||z_i||^2/tau) is computed per own row and subtracted from the
#     row sum (exact to ~1e-4 relative, far below tolerance).
#   - Segment sums (the segment_reduce op): one-hot matmuls over the own
#     shard only -> per-core partial S_e/S_z in class space; host sums
#     the 8 partials.  Counts come from the labels on the host (bincount).
#   - Losses are finished on the host from the small partials (numpy,
#     <10 MFLOP): loss1 from ce_sums/gathered logits, loss2 from
#     prototypes/sse, loss3 from rowsum->lse and ||S_c||^2, loss4 from the
#     prototype Gram matrix.
#
# Output matches reference: tuple (total, loss1, loss2, loss3, loss4) of fp32.

import numpy as np

B = 8192
C = 512  # NUM_CLASSES
D = 256
NCORES = 8
SH = B // NCORES  # 1024 rows per core
T = SH // 128  # 8 row-tiles per core
NT = B // 128  # 64 row-tiles globally
ALPHA = 0.5
BETA = 0.5
GAMMA = 0.5
TAU = 0.1
INV_TAU = 10.0
EPS = 1e-8

_CACHE = {}


def _build():
    import concourse.bass as bass
    import concourse.mybir as mybir
    import concourse.tile as tile
    from concourse import bacc

    f32 = mybir.dt.float32
    f16 = mybir.dt.float16
    i32 = mybir.dt.int32
    AX = mybir.AxisListType
    OP = mybir.AluOpType
    ACT = mybir.ActivationFunctionType

    nc = bacc.Bacc("TRN2", target_bir_lowering=False, debug=False, num_devices=NCORES)

    # Host-packed inputs (all fp16 except labels-as-fp16 already exact ints):
    #   emb_all : [8 chunks, 128 p, 8 t, 256 d]  -- ALL rows, replicated
    #   e_own   : [128 p, 8 t, 256 d]            -- own shard, row-major tiles
    #   logits  : [128 p, 8 t, 512 c]            -- own shard
    #   labels  : [128 p, 8 t]                   -- own shard
    emb_in = nc.dram_tensor("emb_all", [8, 128, 8, D], f16, kind="ExternalInput")
    eo_in = nc.dram_tensor("e_own", [128, T, D], f16, kind="ExternalInput")
    lg_in = nc.dram_tensor("logits", [128, T, C], f16, kind="ExternalInput")
    lab_in = nc.dram_tensor("labels_f", [128, T], f32, kind="ExternalInput")
    seg_out = nc.dram_tensor("seg", [128, 4, C], f32, kind="ExternalOutput")
    stats_out = nc.dram_tensor("stats", [128, 28], f32, kind="ExternalOutput")

    with tile.TileContext(nc) as tc:
        with (
            tc.tile_pool(name="const", bufs=1) as constp,
            tc.tile_pool(name="persist", bufs=1) as pers,
            tc.tile_pool(name="scratch", bufs=3) as scr,
        ):
            # ---------- constants ----------
            iota_i = constp.tile([128, C], i32, name="iota_i")
            nc.gpsimd.iota(iota_i, pattern=[[1, C]], base=0, channel_multiplier=0)
            iota_h = constp.tile([128, C], f16, name="iota_h")
            nc.vector.tensor_copy(iota_h, iota_i)

            lab = constp.tile([128, T], f32, name="lab")
            nc.sync.dma_start(lab, lab_in[:, :])

            # ---------- persistent tiles ----------
            e_all = pers.tile([128, NT, D], f16, name="e_all")
            ztf = [pers.tile([128, B], f16, name=f"ztf{h}") for h in range(2)]
            zto = [pers.tile([128, SH], f16, name=f"zto{h}") for h in range(2)]
            e_own = pers.tile([128, T, D], f16, name="e_own")
            z_own = pers.tile([128, T, D], f16, name="z_own")
            lgt = pers.tile([128, T, C], f16, name="lgt")
            O_t = [pers.tile([128, C], f16, name=f"onehot{t}") for t in range(T)]
            ssqs = pers.tile([128, NT], f32, name="ssqs")
            ssq_o = pers.tile([128, T], f32, name="ssq_o")
            zden = pers.tile([128, NT], f32, name="zden")
            zden_o = pers.tile([128, T], f32, name="zden_o")
            rsA = pers.tile([128, T, 4], f32, name="rsA")
            stats = pers.tile([128, 28], f32, name="stats")
            seg_sb = pers.tile([128, 4, C], f32, name="seg_sb")

            # ---------- loads ----------
            nc.sync.dma_start(e_own, eo_in[:, :, :])
            nc.scalar.dma_start(lgt, lg_in[:, :, :])
            for c in range(8):
                nc.sync.dma_start(e_all[:, c * 8:(c + 1) * 8, :], emb_in[c, :, :, :])

            # ---------- own prep: ssq, zden, z_own, one-hots ----------
            for t in range(T):
                sqo = scr.tile([128, D], f16, name="sqo", tag="sq")
                nc.vector.scalar_tensor_tensor(
                    out=sqo, in0=e_own[:, t, :], scalar=1.0, in1=e_own[:, t, :],
                    op0=OP.mult, op1=OP.mult, accum_out=ssq_o[:, t:t + 1],
                )

            def rsqrt_chain(dst, src, width, tagsfx):
                # dst = 1 / (sqrt_newton(src) + EPS)
                n0 = scr.tile([128, width], f32, name="n0" + tagsfx, tag="c0" + tagsfx)
                nc.scalar.activation(n0, src, ACT.Sqrt)
                n0m = scr.tile([128, width], f32, name="n1" + tagsfx, tag="c1" + tagsfx)
                nc.vector.tensor_scalar(n0m, n0, 1e-20, None, OP.max)
                r0 = scr.tile([128, width], f32, name="n2" + tagsfx, tag="c2" + tagsfx)
                nc.vector.reciprocal(r0, n0m)
                t1 = scr.tile([128, width], f32, name="n3" + tagsfx, tag="c3" + tagsfx)
                nc.vector.tensor_tensor(t1, src, r0, OP.mult)
                nc.vector.tensor_tensor(t1, t1, n0m, OP.add)
                nc.vector.tensor_scalar(t1, t1, 0.5, EPS, OP.mult, OP.add)
                nc.vector.reciprocal(dst, t1)

            rsqrt_chain(zden_o, ssq_o, T, "o")
            for t in range(T):
                nc.vector.tensor_scalar(
                    z_own[:, t, :], e_own[:, t, :], zden_o[:, t:t + 1], None, OP.mult
                )
                nc.vector.tensor_scalar(O_t[t], iota_h, lab[:, t:t + 1], None, OP.is_equal)
            # own z^T (static location, diagonal position known per row-tile)
            for t in range(T):
                for h in range(2):
                    eng = nc.sync if (t % 2 == 0) else nc.scalar
                    eng.dma_start_transpose(
                        zto[h][:, t * 128:(t + 1) * 128],
                        z_own[:, t, h * 128:(h + 1) * 128],
                    )

            # ---------- segment sums over own shard (one-hot matmuls) ----------
            with tc.tile_pool(name="segps", bufs=1, space="PSUM") as segpsp:
                segps = [segpsp.tile([128, C], f32, name=f"segps{i}") for i in range(4)]
                for t in range(T):
                    for h in range(2):
                        nc.tensor.matmul(
                            segps[h], e_own[:, t, h * 128:(h + 1) * 128], O_t[t],
                            start=(t == 0), stop=(t == T - 1),
                        )
                        nc.tensor.matmul(
                            segps[2 + h], z_own[:, t, h * 128:(h + 1) * 128], O_t[t],
                            start=(t == 0), stop=(t == T - 1),
                        )
                for i in range(4):
                    nc.vector.tensor_copy(seg_sb[:, i, :], segps[i])
            nc.sync.dma_start(seg_out[:, :, :], seg_sb)

            # sse partial (own shard); gls gather (no ACT involved)
            sse8 = pers.tile([128, T], f32, name="sse8")
            for t in range(T):
                sq2 = scr.tile([128, D], f16, name="sq2", tag="sq")
                nc.vector.scalar_tensor_tensor(
                    out=sq2, in0=e_own[:, t, :], scalar=1.0, in1=e_own[:, t, :],
                    op0=OP.mult, op1=OP.mult, accum_out=sse8[:, t:t + 1],
                )
            nc.vector.tensor_reduce(stats[:, 24:25], sse8, AX.X, OP.add)
            for t in range(T):
                gsc = scr.tile([128, C], f16, name="gsc", tag="gsc")
                nc.vector.scalar_tensor_tensor(
                    out=gsc, in0=O_t[t], scalar=1.0, in1=lgt[:, t, :],
                    op0=OP.mult, op1=OP.mult, accum_out=stats[:, 16 + t:17 + t],
                )

            # ---------- normalize ALL rows, build z^T ----------
            for g in range(8):
                for j in range(g * 8, (g + 1) * 8):
                    sq = scr.tile([128, D], f16, name="sq", tag="sq")
                    nc.vector.scalar_tensor_tensor(
                        out=sq, in0=e_all[:, j, :], scalar=1.0, in1=e_all[:, j, :],
                        op0=OP.mult, op1=OP.mult, accum_out=ssqs[:, j:j + 1],
                    )
                rsqrt_chain(
                    zden[:, g * 8:(g + 1) * 8], ssqs[:, g * 8:(g + 1) * 8], 8, "a"
                )
                for j in range(g * 8, (g + 1) * 8):
                    # scale on Pool (otherwise idle): zden broadcast via
                    # stride-0 free-dim AP
                    zb = zden[:, j:j + 1]
                    zb_b = bass.AP(zb.tensor, zb.offset, [zb.ap[0], [0, D]])
                    nc.gpsimd.tensor_tensor(
                        e_all[:, j, :], e_all[:, j, :], zb_b, OP.mult
                    )
                    for h in range(2):
                        eng = nc.sync if (j % 2 == 0) else nc.scalar
                        eng.dma_start_transpose(
                            ztf[h][:, j * 128:(j + 1) * 128],
                            e_all[:, j, h * 128:(h + 1) * 128],
                        )

            # ---------- CE exp (after all Sqrt chains: one table switch) ----------
            esc = pers.tile([128, T, C], f16, name="esc")
            nc.scalar.activation(esc, lgt, ACT.Exp)
            nc.vector.tensor_reduce(stats[:, 8:16], esc, AX.X, OP.add)

            # ---------- sim rows: exp row-sums ----------
            with tc.tile_pool(name="simps", bufs=2, space="PSUM") as simpsp:
                for jc in range(4):
                    for r in range(T):
                        ps = simpsp.tile([128, 2048], f32, name="ps", tag="ps")
                        for jb in range(4):
                            for h in range(2):
                                nc.tensor.matmul(
                                    ps[:, jb * 512:(jb + 1) * 512],
                                    zto[h][:, r * 128:(r + 1) * 128],
                                    ztf[h][:, jc * 2048 + jb * 512: jc * 2048 + (jb + 1) * 512],
                                    start=(h == 0), stop=(h == 1),
                                )
                        ex = scr.tile([128, 2048], f16, name="ex", tag="ex")
                        nc.scalar.activation(
                            ex, ps, ACT.Exp, scale=INV_TAU,
                            accum_out=rsA[:, r, jc:jc + 1],
                        )

            # diagonal correction: rowsum -= exp(||z_i||^2 / tau)
            zd2 = pers.tile([128, T], f32, name="zd2")
            nc.vector.tensor_tensor(zd2, zden_o, zden_o, OP.mult)
            nc.vector.tensor_tensor(zd2, zd2, ssq_o, OP.mult)
            dexp = pers.tile([128, T], f32, name="dexp")
            nc.scalar.activation(dexp, zd2, ACT.Exp, scale=INV_TAU)
            rsred = pers.tile([128, T], f32, name="rsred")
            nc.vector.tensor_reduce(rsred, rsA, AX.X, OP.add)
            nc.vector.tensor_tensor(stats[:, 0:8], rsred, dexp, OP.subtract)

            nc.vector.memset(stats[:, 25:28], 0.0)
            nc.sync.dma_start(stats_out[:, :], stats)

    nc.compile()
    return nc


def _get_nc():
    if "nc" not in _CACHE:
        _CACHE["nc"] = _build()
    return _CACHE["nc"]


def _pack_inputs(logits, embeddings, labels):
    """Host-side sharding / layout packing (fp16 casts + tile packing)."""
    e16 = embeddings.astype(np.float16)          # [8192, 256]
    lg16 = logits.astype(np.float16)             # [8192, 512]
    # emb_all: [8 chunks, 128 p, 8 t, 256 d], chunk c tile t = global tile c*8+t
    emb_all = np.ascontiguousarray(
        e16.reshape(8, 8, 128, D).transpose(0, 2, 1, 3)
    )
    in_maps = []
    for c in range(NCORES):
        sl = slice(c * SH, (c + 1) * SH)
        eo = np.ascontiguousarray(e16[sl].reshape(T, 128, D).transpose(1, 0, 2))
        lg = np.ascontiguousarray(lg16[sl].reshape(T, 128, C).transpose(1, 0, 2))
        lab = np.ascontiguousarray(
            labels[sl].astype(np.float32).reshape(T, 128).T
        )
        in_maps.append({
            "emb_all": emb_all,
            "e_own": eo,
            "logits": lg,
            "labels_f": lab,
        })
    return in_maps


def _finish(results, labels):
    """Combine per-core partials into the five losses (host, numpy)."""
    labels = np.asarray(labels).astype(np.int64)
    counts = np.bincount(labels, minlength=C).astype(np.float64)

    seg = np.zeros((128, 4, C), np.float64)
    rowsums = np.zeros(B, np.float64)
    ce_sums = np.zeros(B, np.float64)
    gls = np.zeros(B, np.float64)
    sse = 0.0
    for c in range(NCORES):
        r = results[c]
        seg += r["seg"].astype(np.float64)
        st = r["stats"].astype(np.float64)
        sl = slice(c * SH, (c + 1) * SH)
        # stats[:, t] covers global rows c*1024 + t*128 + p  (p = partition)
        rowsums[sl] = st[:, 0:8].T.reshape(-1)
        ce_sums[sl] = st[:, 8:16].T.reshape(-1)
        gls[sl] = st[:, 16:24].T.reshape(-1)
        sse += st[:, 24].sum()

    # seg[p, i, c]: i in {e_lo, e_hi, z_lo, z_hi}; d = (i%2)*128 + p
    S_e = np.concatenate([seg[:, 0, :], seg[:, 1, :]], axis=0)  # [256, 512] (d, c)
    S_z = np.concatenate([seg[:, 2, :], seg[:, 3, :]], axis=0)

    cntm = np.maximum(counts, 1.0)
    protos = (S_e / cntm).T  # [512, 256]

    # loss1: cross-entropy
    l1 = float(np.mean(np.log(ce_sums) - gls))

    # loss2: assignment loss
    l2 = float((sse - np.sum(counts * np.sum(protos * protos, axis=1))) / B)

    # loss3: SupCon
    lse = np.log(rowsums)
    v2 = counts >= 2.0
    t3b = float(lse[v2[labels]].sum())
    Sz2 = np.sum(S_z * S_z, axis=0)  # ||S_c||^2
    cm1 = np.maximum(counts - 1.0, 1.0)
    t3a = float(np.sum((Sz2 - counts)[v2] * INV_TAU / cm1[v2]))
    nvalid = float(counts[v2].sum())
    l3 = -(t3a - t3b) / max(nvalid, 1.0)

    # loss4: prototype orthogonality
    pnorm = np.sqrt(np.sum(protos * protos, axis=1))
    pn = protos / (pnorm + EPS)[:, None]
    present = counts > 0
    Psub = pn[present].astype(np.float64)
    G = Psub @ Psub.T
    npres = float(present.sum())
    l4 = float((np.sum(G * G) - np.sum(np.diag(G) ** 2))
               / max(npres * npres - npres, 1.0))

    total = l1 + ALPHA * l2 + BETA * l3 + GAMMA * l4
    return tuple(np.float32(v) for v in (total, l1, l2, l3, l4))


def kernel(logits, embeddings, labels):
    from concourse import bass_utils

    nc = _get_nc()
    logits = np.ascontiguousarray(np.asarray(logits, dtype=np.float32))
    embeddings = np.ascontiguousarray(np.asarray(embeddings, dtype=np.float32))
    labels_np = np.asarray(labels)

    in_maps = _pack_inputs(logits, embeddings, labels_np)
    res = bass_utils.run_bass_kernel_spmd(nc, in_maps, core_ids=list(range(NCORES)))
    return _finish(res.results, labels_np)


# revision 14
# speedup vs baseline: 40.1707x; 1.3039x over previous
# Trainium2 Bass kernel for nn_CombinedLoss (CE + proto-assignment + SupCon + proto-orthogonality)
#
# Strategy (8 NeuronCores, data-parallel over batch, COLLECTIVE-FREE):
#   - Each core gets its own 1024-row shard of logits/labels/embeddings
#     (for CE, segment sums, and the sim-matrix rows) plus a replicated
#     copy of ALL embeddings (fp16) so it can build the full z^T [256,8192]
#     locally.  No AllGather / AllReduce: cross-core combination of the
#     small per-core partials (segment sums [512,512], row-sums [1024],
#     CE pieces) happens on the host, so the 8 cores never synchronize
#     with each other on device.  This removes the collective stalls that
#     dominated the previous version (each exec forced an 8-core
#     rendezvous; the dispatch skew was absorbed as on-device wait).
#   - z^T is built on-device: square-accumulate (Pool engine), a
#     Newton-refined rsqrt chain, scale (DVE), then SBUF->SBUF DMA
#     transposes (XBAR) -- no PE transposes, no PSUM traffic.
#   - Sim rows: out = z_ownT.T @ z_allT in fp16 (PE), exp+row-accumulate
#     on the Activation engine.  The diagonal is NOT masked on-device;
#     exp(||z_i||^2/tau) is computed per own row and subtracted from the
#     row sum (exact to ~1e-4 relative, far below tolerance).
#   - Segment sums (the segment_reduce op): one-hot matmuls over the own
#     shard only -> per-core partial S_e/S_z in class space; host sums
#     the 8 partials.  Counts come from the labels on the host (bincount).
#   - Losses are finished on the host from the small partials (numpy,
#     <10 MFLOP): loss1 from ce_sums/gathered logits, loss2 from
#     prototypes/sse, loss3 from rowsum->lse and ||S_c||^2, loss4 from the
#     prototype Gram matrix.
#
# Output matches reference: tuple (total, loss1, loss2, loss3, loss4) of fp32.

import numpy as np

B = 8192
C = 512  # NUM_CLASSES
D = 256
NCORES = 8
SH = B // NCORES  # 1024 rows per core
T = SH // 128  # 8 row-tiles per core
NT = B // 128  # 64 row-tiles globally
ALPHA = 0.5
BETA = 0.5
GAMMA = 0.5
TAU = 0.1
INV_TAU = 10.0
EPS = 1e-8

_CACHE = {}


def _build():
    import concourse.bass as bass
    import concourse.mybir as mybir
    import concourse.tile as tile
    from concourse import bacc

    f32 = mybir.dt.float32
    f16 = mybir.dt.float16
    i32 = mybir.dt.int32
    AX = mybir.AxisListType
    OP = mybir.AluOpType
    ACT = mybir.ActivationFunctionType

    nc = bacc.Bacc("TRN2", target_bir_lowering=False, debug=False, num_devices=NCORES)

    # Host-packed inputs, consolidated to 2 tensors/core (fewer PJRT buffers
    # = lower per-exec dispatch cost through the axon tunnel):
    #   emb_all : [8 chunks, 128 p, 8 t, 256 d] fp16 -- ALL rows, replicated
    #   own     : [128 p, 8 t, 769] fp16 -- own shard: logits(512) |
    #             embeddings(256) | label(1; ints exact in fp16)
    # One output: out[128, 2076] = seg partials (4*512) | stats (28)
    emb_in = nc.dram_tensor("emb_all", [8, 128, 8, D], f16, kind="ExternalInput")
    own_in = nc.dram_tensor("own", [128, T, C + D + 1], f16, kind="ExternalInput")
    out_dram = nc.dram_tensor("out", [128, 4 * C + 28], f32, kind="ExternalOutput")

    with tile.TileContext(nc) as tc:
        with (
            tc.tile_pool(name="const", bufs=1) as constp,
            tc.tile_pool(name="persist", bufs=1) as pers,
            tc.tile_pool(name="scratch", bufs=3) as scr,
        ):
            # ---------- constants ----------
            iota_i = constp.tile([128, C], i32, name="iota_i")
            nc.gpsimd.iota(iota_i, pattern=[[1, C]], base=0, channel_multiplier=0)
            iota_h = constp.tile([128, C], f16, name="iota_h")
            nc.vector.tensor_copy(iota_h, iota_i)

            lab16 = constp.tile([128, T], f16, name="lab16")
            nc.sync.dma_start(lab16, own_in[:, :, C + D])
            lab = constp.tile([128, T], f32, name="lab")
            nc.vector.tensor_copy(lab, lab16)

            # ---------- persistent tiles ----------
            e_all = pers.tile([128, NT, D], f16, name="e_all")
            ztf = [pers.tile([128, B], f16, name=f"ztf{h}") for h in range(2)]
            zto = [pers.tile([128, SH], f16, name=f"zto{h}") for h in range(2)]
            e_own = pers.tile([128, T, D], f16, name="e_own")
            z_own = pers.tile([128, T, D], f16, name="z_own")
            lgt = pers.tile([128, T, C], f16, name="lgt")
            O_t = [pers.tile([128, C], f16, name=f"onehot{t}") for t in range(T)]
            ssqs = pers.tile([128, NT], f32, name="ssqs")
            ssq_o = pers.tile([128, T], f32, name="ssq_o")
            zden = pers.tile([128, NT], f32, name="zden")
            zden_o = pers.tile([128, T], f32, name="zden_o")
            rsA = pers.tile([128, T, 4], f32, name="rsA")
            stats = pers.tile([128, 28], f32, name="stats")
            seg_sb = pers.tile([128, 4, C], f32, name="seg_sb")

            # ---------- loads ----------
            nc.sync.dma_start(e_own, own_in[:, :, C:C + D])
            nc.scalar.dma_start(lgt, own_in[:, :, 0:C])
            for c in range(8):
                nc.sync.dma_start(e_all[:, c * 8:(c + 1) * 8, :], emb_in[c, :, :, :])

            # ---------- own prep: ssq, zden, z_own, one-hots ----------
            for t in range(T):
                sqo = scr.tile([128, D], f16, name="sqo", tag="sq")
                nc.vector.scalar_tensor_tensor(
                    out=sqo, in0=e_own[:, t, :], scalar=1.0, in1=e_own[:, t, :],
                    op0=OP.mult, op1=OP.mult, accum_out=ssq_o[:, t:t + 1],
                )

            def rsqrt_chain(dst, src, width, tagsfx):
                # dst = 1 / (sqrt_newton(src) + EPS)
                n0 = scr.tile([128, width], f32, name="n0" + tagsfx, tag="c0" + tagsfx)
                nc.scalar.activation(n0, src, ACT.Sqrt)
                n0m = scr.tile([128, width], f32, name="n1" + tagsfx, tag="c1" + tagsfx)
                nc.vector.tensor_scalar(n0m, n0, 1e-20, None, OP.max)
                r0 = scr.tile([128, width], f32, name="n2" + tagsfx, tag="c2" + tagsfx)
                nc.vector.reciprocal(r0, n0m)
                t1 = scr.tile([128, width], f32, name="n3" + tagsfx, tag="c3" + tagsfx)
                nc.vector.tensor_tensor(t1, src, r0, OP.mult)
                nc.vector.tensor_tensor(t1, t1, n0m, OP.add)
                nc.vector.tensor_scalar(t1, t1, 0.5, EPS, OP.mult, OP.add)
                nc.vector.reciprocal(dst, t1)

            rsqrt_chain(zden_o, ssq_o, T, "o")
            for t in range(T):
                nc.vector.tensor_scalar(
                    z_own[:, t, :], e_own[:, t, :], zden_o[:, t:t + 1], None, OP.mult
                )
                nc.vector.tensor_scalar(O_t[t], iota_h, lab[:, t:t + 1], None, OP.is_equal)
            # own z^T (static location, diagonal position known per row-tile)
            for t in range(T):
                for h in range(2):
                    eng = nc.sync if (t % 2 == 0) else nc.scalar
                    eng.dma_start_transpose(
                        zto[h][:, t * 128:(t + 1) * 128],
                        z_own[:, t, h * 128:(h + 1) * 128],
                    )

            # ---------- segment sums over own shard (one-hot matmuls) ----------
            with tc.tile_pool(name="segps", bufs=1, space="PSUM") as segpsp:
                segps = [segpsp.tile([128, C], f32, name=f"segps{i}") for i in range(4)]
                for t in range(T):
                    for h in range(2):
                        nc.tensor.matmul(
                            segps[h], e_own[:, t, h * 128:(h + 1) * 128], O_t[t],
                            start=(t == 0), stop=(t == T - 1),
                        )
                        nc.tensor.matmul(
                            segps[2 + h], z_own[:, t, h * 128:(h + 1) * 128], O_t[t],
                            start=(t == 0), stop=(t == T - 1),
                        )
                for i in range(4):
                    nc.vector.tensor_copy(seg_sb[:, i, :], segps[i])
            nc.sync.dma_start(out_dram[:, 0:4 * C], seg_sb[:, :, :].opt())

            # sse partial (own shard); gls gather (no ACT involved)
            sse8 = pers.tile([128, T], f32, name="sse8")
            for t in range(T):
                sq2 = scr.tile([128, D], f16, name="sq2", tag="sq")
                nc.vector.scalar_tensor_tensor(
                    out=sq2, in0=e_own[:, t, :], scalar=1.0, in1=e_own[:, t, :],
                    op0=OP.mult, op1=OP.mult, accum_out=sse8[:, t:t + 1],
                )
            nc.vector.tensor_reduce(stats[:, 24:25], sse8, AX.X, OP.add)
            for t in range(T):
                gsc = scr.tile([128, C], f16, name="gsc", tag="gsc")
                nc.vector.scalar_tensor_tensor(
                    out=gsc, in0=O_t[t], scalar=1.0, in1=lgt[:, t, :],
                    op0=OP.mult, op1=OP.mult, accum_out=stats[:, 16 + t:17 + t],
                )

            # ---------- normalize ALL rows, build z^T ----------
            for g in range(8):
                for j in range(g * 8, (g + 1) * 8):
                    sq = scr.tile([128, D], f16, name="sq", tag="sq")
                    nc.vector.scalar_tensor_tensor(
                        out=sq, in0=e_all[:, j, :], scalar=1.0, in1=e_all[:, j, :],
                        op0=OP.mult, op1=OP.mult, accum_out=ssqs[:, j:j + 1],
                    )
                rsqrt_chain(
                    zden[:, g * 8:(g + 1) * 8], ssqs[:, g * 8:(g + 1) * 8], 8, "a"
                )
                for j in range(g * 8, (g + 1) * 8):
                    # scale on Pool (otherwise idle): zden broadcast via
                    # stride-0 free-dim AP
                    zb = zden[:, j:j + 1]
                    zb_b = bass.AP(zb.tensor, zb.offset, [zb.ap[0], [0, D]])
                    nc.gpsimd.tensor_tensor(
                        e_all[:, j, :], e_all[:, j, :], zb_b, OP.mult
                    )
                    for h in range(2):
                        eng = nc.sync if (j % 2 == 0) else nc.scalar
                        eng.dma_start_transpose(
                            ztf[h][:, j * 128:(j + 1) * 128],
                            e_all[:, j, h * 128:(h + 1) * 128],
                        )

            # ---------- CE exp (after all Sqrt chains: one table switch) ----------
            esc = pers.tile([128, T, C], f16, name="esc")
            nc.scalar.activation(esc, lgt, ACT.Exp)
            nc.vector.tensor_reduce(stats[:, 8:16], esc, AX.X, OP.add)

            # ---------- sim rows: exp row-sums ----------
            with tc.tile_pool(name="simps", bufs=2, space="PSUM") as simpsp:
                for jc in range(4):
                    for r in range(T):
                        ps = simpsp.tile([128, 2048], f32, name="ps", tag="ps")
                        for jb in range(4):
                            for h in range(2):
                                nc.tensor.matmul(
                                    ps[:, jb * 512:(jb + 1) * 512],
                                    zto[h][:, r * 128:(r + 1) * 128],
                                    ztf[h][:, jc * 2048 + jb * 512: jc * 2048 + (jb + 1) * 512],
                                    start=(h == 0), stop=(h == 1),
                                )
                        ex = scr.tile([128, 2048], f16, name="ex", tag="ex")
                        nc.scalar.activation(
                            ex, ps, ACT.Exp, scale=INV_TAU,
                            accum_out=rsA[:, r, jc:jc + 1],
                        )

            # diagonal correction: rowsum -= exp(||z_i||^2 / tau)
            zd2 = pers.tile([128, T], f32, name="zd2")
            nc.vector.tensor_tensor(zd2, zden_o, zden_o, OP.mult)
            nc.vector.tensor_tensor(zd2, zd2, ssq_o, OP.mult)
            dexp = pers.tile([128, T], f32, name="dexp")
            nc.scalar.activation(dexp, zd2, ACT.Exp, scale=INV_TAU)
            rsred = pers.tile([128, T], f32, name="rsred")
            nc.vector.tensor_reduce(rsred, rsA, AX.X, OP.add)
            nc.vector.tensor_tensor(stats[:, 0:8], rsred, dexp, OP.subtract)

            nc.vector.memset(stats[:, 25:28], 0.0)
            nc.sync.dma_start(out_dram[:, 4 * C:4 * C + 28], stats)

    nc.compile()
    return nc


def _get_nc():
    if "nc" not in _CACHE:
        _CACHE["nc"] = _build()
    return _CACHE["nc"]


def _pack_inputs(logits, embeddings, labels):
    """Host-side sharding / layout packing (fp16 casts + tile packing)."""
    e16 = embeddings.astype(np.float16)          # [8192, 256]
    lg16 = logits.astype(np.float16)             # [8192, 512]
    # emb_all: [8 chunks, 128 p, 8 t, 256 d], chunk c tile t = global tile c*8+t
    emb_all = np.ascontiguousarray(
        e16.reshape(8, 8, 128, D).transpose(0, 2, 1, 3)
    )
    lab16 = labels.astype(np.float16)
    in_maps = []
    for c in range(NCORES):
        sl = slice(c * SH, (c + 1) * SH)
        own = np.empty((128, T, C + D + 1), np.float16)
        own[:, :, 0:C] = lg16[sl].reshape(T, 128, C).transpose(1, 0, 2)
        own[:, :, C:C + D] = e16[sl].reshape(T, 128, D).transpose(1, 0, 2)
        own[:, :, C + D] = lab16[sl].reshape(T, 128).T
        in_maps.append({"emb_all": emb_all, "own": own})
    return in_maps


def _finish(results, labels):
    """Combine per-core partials into the five losses (host, numpy)."""
    labels = np.asarray(labels).astype(np.int64)
    counts = np.bincount(labels, minlength=C).astype(np.float64)

    seg = np.zeros((128, 4, C), np.float64)
    rowsums = np.zeros(B, np.float64)
    ce_sums = np.zeros(B, np.float64)
    gls = np.zeros(B, np.float64)
    sse = 0.0
    for c in range(NCORES):
        o = results[c]["out"].astype(np.float64)
        seg += o[:, 0:4 * C].reshape(128, 4, C)
        st = o[:, 4 * C:4 * C + 28]
        sl = slice(c * SH, (c + 1) * SH)
        # stats[:, t] covers global rows c*1024 + t*128 + p  (p = partition)
        rowsums[sl] = st[:, 0:8].T.reshape(-1)
        ce_sums[sl] = st[:, 8:16].T.reshape(-1)
        gls[sl] = st[:, 16:24].T.reshape(-1)
        sse += st[:, 24].sum()

    # seg[p, i, c]: i in {e_lo, e_hi, z_lo, z_hi}; d = (i%2)*128 + p
    S_e = np.concatenate([seg[:, 0, :], seg[:, 1, :]], axis=0)  # [256, 512] (d, c)
    S_z = np.concatenate([seg[:, 2, :], seg[:, 3, :]], axis=0)

    cntm = np.maximum(counts, 1.0)
    protos = (S_e / cntm).T  # [512, 256]

    # loss1: cross-entropy
    l1 = float(np.mean(np.log(ce_sums) - gls))

    # loss2: assignment loss
    l2 = float((sse - np.sum(counts * np.sum(protos * protos, axis=1))) / B)

    # loss3: SupCon
    lse = np.log(rowsums)
    v2 = counts >= 2.0
    t3b = float(lse[v2[labels]].sum())
    Sz2 = np.sum(S_z * S_z, axis=0)  # ||S_c||^2
    cm1 = np.maximum(counts - 1.0, 1.0)
    t3a = float(np.sum((Sz2 - counts)[v2] * INV_TAU / cm1[v2]))
    nvalid = float(counts[v2].sum())
    l3 = -(t3a - t3b) / max(nvalid, 1.0)

    # loss4: prototype orthogonality
    pnorm = np.sqrt(np.sum(protos * protos, axis=1))
    pn = protos / (pnorm + EPS)[:, None]
    present = counts > 0
    Psub = pn[present].astype(np.float64)
    G = Psub @ Psub.T
    npres = float(present.sum())
    l4 = float((np.sum(G * G) - np.sum(np.diag(G) ** 2))
               / max(npres * npres - npres, 1.0))

    total = l1 + ALPHA * l2 + BETA * l3 + GAMMA * l4
    return tuple(np.float32(v) for v in (total, l1, l2, l3, l4))


def kernel(logits, embeddings, labels):
    from concourse import bass_utils

    nc = _get_nc()
    logits = np.ascontiguousarray(np.asarray(logits, dtype=np.float32))
    embeddings = np.ascontiguousarray(np.asarray(embeddings, dtype=np.float32))
    labels_np = np.asarray(labels)

    in_maps = _pack_inputs(logits, embeddings, labels_np)
    res = bass_utils.run_bass_kernel_spmd(nc, in_maps, core_ids=list(range(NCORES)))
    return _finish(res.results, labels_np)


# revision 15
# speedup vs baseline: 44.8375x; 1.1162x over previous
# Trainium2 Bass kernel for nn_CombinedLoss (CE + proto-assignment + SupCon + proto-orthogonality)
#
# Strategy (8 NeuronCores, data-parallel over batch, COLLECTIVE-FREE):
#   - Each core gets its own 1024-row shard of logits/labels/embeddings
#     (for CE, segment sums, and the sim-matrix rows) plus a replicated
#     copy of ALL embeddings (fp16) so it can build the full z^T [256,8192]
#     locally.  No AllGather / AllReduce: cross-core combination of the
#     small per-core partials (segment sums [512,512], row-sums [1024],
#     CE pieces) happens on the host, so the 8 cores never synchronize
#     with each other on device.  This removes the collective stalls that
#     dominated the previous version (each exec forced an 8-core
#     rendezvous; the dispatch skew was absorbed as on-device wait).
#   - z^T is built on-device: square-accumulate (Pool engine), a
#     Newton-refined rsqrt chain, scale (DVE), then SBUF->SBUF DMA
#     transposes (XBAR) -- no PE transposes, no PSUM traffic.
#   - Sim rows: out = z_ownT.T @ z_allT in fp16 (PE), exp+row-accumulate
#     on the Activation engine.  The diagonal is NOT masked on-device;
#     exp(||z_i||^2/tau) is computed per own row and subtracted from the
#     row sum (exact to ~1e-4 relative, far below tolerance).
#   - Segment sums (the segment_reduce op): one-hot matmuls over the own
#     shard only -> per-core partial S_e/S_z in class space; host sums
#     the 8 partials.  Counts come from the labels on the host (bincount).
#   - Losses are finished on the host from the small partials (numpy,
#     <10 MFLOP): loss1 from ce_sums/gathered logits, loss2 from
#     prototypes/sse, loss3 from rowsum->lse and ||S_c||^2, loss4 from the
#     prototype Gram matrix.
#
# Output matches reference: tuple (total, loss1, loss2, loss3, loss4) of fp32.

import numpy as np

B = 8192
C = 512  # NUM_CLASSES
D = 256
NCORES = 8
SH = B // NCORES  # 1024 rows per core
T = SH // 128  # 8 row-tiles per core
NT = B // 128  # 64 row-tiles globally
ALPHA = 0.5
BETA = 0.5
GAMMA = 0.5
TAU = 0.1
INV_TAU = 10.0
EPS = 1e-8

_CACHE = {}


def _build():
    import concourse.bass as bass
    import concourse.mybir as mybir
    import concourse.tile as tile
    from concourse import bacc

    f32 = mybir.dt.float32
    f16 = mybir.dt.float16
    i32 = mybir.dt.int32
    AX = mybir.AxisListType
    OP = mybir.AluOpType
    ACT = mybir.ActivationFunctionType

    nc = bacc.Bacc("TRN2", target_bir_lowering=False, debug=False, num_devices=NCORES)

    # Host-packed inputs, consolidated to 2 tensors/core (fewer PJRT buffers
    # = lower per-exec dispatch cost through the axon tunnel):
    #   emb_all : [8 chunks, 128 p, 8 t, 256 d] fp16 -- ALL rows, replicated
    #   own     : [128 p, 8 t, 769] fp16 -- own shard: logits(512) |
    #             embeddings(256) | label(1; ints exact in fp16)
    # One output: out[128, 2076] = seg partials (4*512) | stats (28)
    emb_in = nc.dram_tensor("emb_all", [8, 128, 8, D], f16, kind="ExternalInput")
    own_in = nc.dram_tensor("own", [128, T, C + D + 1], f16, kind="ExternalInput")
    out_dram = nc.dram_tensor("out", [128, 4 * C + 28], f32, kind="ExternalOutput")

    with tile.TileContext(nc) as tc:
        with (
            tc.tile_pool(name="const", bufs=1) as constp,
            tc.tile_pool(name="persist", bufs=1) as pers,
            tc.tile_pool(name="scratch", bufs=3) as scr,
        ):
            # ---------- constants ----------
            iota_i = constp.tile([128, C], i32, name="iota_i")
            nc.gpsimd.iota(iota_i, pattern=[[1, C]], base=0, channel_multiplier=0)
            iota_h = constp.tile([128, C], f16, name="iota_h")
            nc.vector.tensor_copy(iota_h, iota_i)

            lab16 = constp.tile([128, T], f16, name="lab16")
            nc.sync.dma_start(lab16, own_in[:, :, C + D])
            lab = constp.tile([128, T], f32, name="lab")
            nc.vector.tensor_copy(lab, lab16)

            # ---------- persistent tiles ----------
            e_all = pers.tile([128, NT, D], f16, name="e_all")
            ztf = [pers.tile([128, B], f16, name=f"ztf{h}") for h in range(2)]
            zto = [pers.tile([128, SH], f16, name=f"zto{h}") for h in range(2)]
            e_own = pers.tile([128, T, D], f16, name="e_own")
            z_own = pers.tile([128, T, D], f16, name="z_own")
            lgt = pers.tile([128, T, C], f16, name="lgt")
            O_t = [pers.tile([128, C], f16, name=f"onehot{t}") for t in range(T)]
            ssqs = pers.tile([128, NT], f32, name="ssqs")
            ssq_o = pers.tile([128, T], f32, name="ssq_o")
            zden = pers.tile([128, NT], f32, name="zden")
            zden_o = pers.tile([128, T], f32, name="zden_o")
            rsA = pers.tile([128, T, 4], f32, name="rsA")
            stats = pers.tile([128, 28], f32, name="stats")
            seg_sb = pers.tile([128, 4, C], f32, name="seg_sb")

            # ---------- loads ----------
            nc.sync.dma_start(e_own, own_in[:, :, C:C + D])
            nc.scalar.dma_start(lgt, own_in[:, :, 0:C])
            for c in range(8):
                nc.sync.dma_start(e_all[:, c * 8:(c + 1) * 8, :], emb_in[c, :, :, :])

            # ---------- own prep: ssq, zden, z_own, one-hots ----------
            for t in range(T):
                sqo = scr.tile([128, D], f16, name="sqo", tag="sq")
                nc.vector.scalar_tensor_tensor(
                    out=sqo, in0=e_own[:, t, :], scalar=1.0, in1=e_own[:, t, :],
                    op0=OP.mult, op1=OP.mult, accum_out=ssq_o[:, t:t + 1],
                )

            def rsqrt_chain(dst, src, width, tagsfx):
                # dst = 1 / (sqrt_newton(src) + EPS)
                n0 = scr.tile([128, width], f32, name="n0" + tagsfx, tag="c0" + tagsfx)
                nc.scalar.activation(n0, src, ACT.Sqrt)
                n0m = scr.tile([128, width], f32, name="n1" + tagsfx, tag="c1" + tagsfx)
                nc.vector.tensor_scalar(n0m, n0, 1e-20, None, OP.max)
                r0 = scr.tile([128, width], f32, name="n2" + tagsfx, tag="c2" + tagsfx)
                nc.vector.reciprocal(r0, n0m)
                t1 = scr.tile([128, width], f32, name="n3" + tagsfx, tag="c3" + tagsfx)
                nc.vector.tensor_tensor(t1, src, r0, OP.mult)
                nc.vector.tensor_tensor(t1, t1, n0m, OP.add)
                nc.vector.tensor_scalar(t1, t1, 0.5, EPS, OP.mult, OP.add)
                nc.vector.reciprocal(dst, t1)

            rsqrt_chain(zden_o, ssq_o, T, "o")
            for t in range(T):
                nc.vector.tensor_scalar(
                    z_own[:, t, :], e_own[:, t, :], zden_o[:, t:t + 1], None, OP.mult
                )
                nc.vector.tensor_scalar(O_t[t], iota_h, lab[:, t:t + 1], None, OP.is_equal)
            # own z^T (static location, diagonal position known per row-tile)
            for t in range(T):
                for h in range(2):
                    eng = nc.sync if (t % 2 == 0) else nc.scalar
                    eng.dma_start_transpose(
                        zto[h][:, t * 128:(t + 1) * 128],
                        z_own[:, t, h * 128:(h + 1) * 128],
                    )

            # ---------- segment sums over own shard (one-hot matmuls) ----------
            with tc.tile_pool(name="segps", bufs=1, space="PSUM") as segpsp:
                segps = [segpsp.tile([128, C], f32, name=f"segps{i}") for i in range(4)]
                for t in range(T):
                    for h in range(2):
                        nc.tensor.matmul(
                            segps[h], e_own[:, t, h * 128:(h + 1) * 128], O_t[t],
                            start=(t == 0), stop=(t == T - 1),
                        )
                        nc.tensor.matmul(
                            segps[2 + h], z_own[:, t, h * 128:(h + 1) * 128], O_t[t],
                            start=(t == 0), stop=(t == T - 1),
                        )
                for i in range(4):
                    nc.vector.tensor_copy(seg_sb[:, i, :], segps[i])
            nc.sync.dma_start(out_dram[:, 0:4 * C], seg_sb[:, :, :].opt())

            # sse partial (own shard); gls gather (no ACT involved)
            sse8 = pers.tile([128, T], f32, name="sse8")
            for t in range(T):
                sq2 = scr.tile([128, D], f16, name="sq2", tag="sq")
                nc.vector.scalar_tensor_tensor(
                    out=sq2, in0=e_own[:, t, :], scalar=1.0, in1=e_own[:, t, :],
                    op0=OP.mult, op1=OP.mult, accum_out=sse8[:, t:t + 1],
                )
            nc.vector.tensor_reduce(stats[:, 24:25], sse8, AX.X, OP.add)
            for t in range(T):
                gsc = scr.tile([128, C], f16, name="gsc", tag="gsc")
                nc.vector.scalar_tensor_tensor(
                    out=gsc, in0=O_t[t], scalar=1.0, in1=lgt[:, t, :],
                    op0=OP.mult, op1=OP.mult, accum_out=stats[:, 16 + t:17 + t],
                )

            # ---------- normalize ALL rows, build z^T ----------
            for g in range(8):
                for j in range(g * 8, (g + 1) * 8):
                    sq = scr.tile([128, D], f16, name="sq", tag="sq")
                    nc.vector.scalar_tensor_tensor(
                        out=sq, in0=e_all[:, j, :], scalar=1.0, in1=e_all[:, j, :],
                        op0=OP.mult, op1=OP.mult, accum_out=ssqs[:, j:j + 1],
                    )
                rsqrt_chain(
                    zden[:, g * 8:(g + 1) * 8], ssqs[:, g * 8:(g + 1) * 8], 8, "a"
                )
                for j in range(g * 8, (g + 1) * 8):
                    nc.vector.tensor_scalar(
                        e_all[:, j, :], e_all[:, j, :], zden[:, j:j + 1], None, OP.mult
                    )
                    for h in range(2):
                        eng = nc.sync if (j % 2 == 0) else nc.scalar
                        eng.dma_start_transpose(
                            ztf[h][:, j * 128:(j + 1) * 128],
                            e_all[:, j, h * 128:(h + 1) * 128],
                        )

            # ---------- CE exp (after all Sqrt chains: one table switch) ----------
            esc = pers.tile([128, T, C], f16, name="esc")
            nc.scalar.activation(esc, lgt, ACT.Exp)
            nc.vector.tensor_reduce(stats[:, 8:16], esc, AX.X, OP.add)

            # ---------- sim rows: exp row-sums ----------
            with tc.tile_pool(name="simps", bufs=2, space="PSUM") as simpsp:
                for jc in range(4):
                    for r in range(T):
                        ps = simpsp.tile([128, 2048], f32, name="ps", tag="ps")
                        for jb in range(4):
                            for h in range(2):
                                nc.tensor.matmul(
                                    ps[:, jb * 512:(jb + 1) * 512],
                                    zto[h][:, r * 128:(r + 1) * 128],
                                    ztf[h][:, jc * 2048 + jb * 512: jc * 2048 + (jb + 1) * 512],
                                    start=(h == 0), stop=(h == 1),
                                )
                        ex = scr.tile([128, 2048], f16, name="ex", tag="ex")
                        nc.scalar.activation(
                            ex, ps, ACT.Exp, scale=INV_TAU,
                            accum_out=rsA[:, r, jc:jc + 1],
                        )

            # diagonal correction: rowsum -= exp(||z_i||^2 / tau)
            zd2 = pers.tile([128, T], f32, name="zd2")
            nc.vector.tensor_tensor(zd2, zden_o, zden_o, OP.mult)
            nc.vector.tensor_tensor(zd2, zd2, ssq_o, OP.mult)
            dexp = pers.tile([128, T], f32, name="dexp")
            nc.scalar.activation(dexp, zd2, ACT.Exp, scale=INV_TAU)
            rsred = pers.tile([128, T], f32, name="rsred")
            nc.vector.tensor_reduce(rsred, rsA, AX.X, OP.add)
            nc.vector.tensor_tensor(stats[:, 0:8], rsred, dexp, OP.subtract)

            nc.vector.memset(stats[:, 25:28], 0.0)
            nc.sync.dma_start(out_dram[:, 4 * C:4 * C + 28], stats)

    nc.compile()
    return nc


def _get_nc():
    if "nc" not in _CACHE:
        _CACHE["nc"] = _build()
    return _CACHE["nc"]


def _pack_inputs(logits, embeddings, labels):
    """Host-side sharding / layout packing (fp16 casts + tile packing)."""
    e16 = embeddings.astype(np.float16)          # [8192, 256]
    lg16 = logits.astype(np.float16)             # [8192, 512]
    # emb_all: [8 chunks, 128 p, 8 t, 256 d], chunk c tile t = global tile c*8+t
    emb_all = np.ascontiguousarray(
        e16.reshape(8, 8, 128, D).transpose(0, 2, 1, 3)
    )
    lab16 = labels.astype(np.float16)
    in_maps = []
    for c in range(NCORES):
        sl = slice(c * SH, (c + 1) * SH)
        own = np.empty((128, T, C + D + 1), np.float16)
        own[:, :, 0:C] = lg16[sl].reshape(T, 128, C).transpose(1, 0, 2)
        own[:, :, C:C + D] = e16[sl].reshape(T, 128, D).transpose(1, 0, 2)
        own[:, :, C + D] = lab16[sl].reshape(T, 128).T
        in_maps.append({"emb_all": emb_all, "own": own})
    return in_maps


def _finish(results, labels):
    """Combine per-core partials into the five losses (host, numpy)."""
    labels = np.asarray(labels).astype(np.int64)
    counts = np.bincount(labels, minlength=C).astype(np.float64)

    seg = np.zeros((128, 4, C), np.float64)
    rowsums = np.zeros(B, np.float64)
    ce_sums = np.zeros(B, np.float64)
    gls = np.zeros(B, np.float64)
    sse = 0.0
    for c in range(NCORES):
        o = results[c]["out"].astype(np.float64)
        seg += o[:, 0:4 * C].reshape(128, 4, C)
        st = o[:, 4 * C:4 * C + 28]
        sl = slice(c * SH, (c + 1) * SH)
        # stats[:, t] covers global rows c*1024 + t*128 + p  (p = partition)
        rowsums[sl] = st[:, 0:8].T.reshape(-1)
        ce_sums[sl] = st[:, 8:16].T.reshape(-1)
        gls[sl] = st[:, 16:24].T.reshape(-1)
        sse += st[:, 24].sum()

    # seg[p, i, c]: i in {e_lo, e_hi, z_lo, z_hi}; d = (i%2)*128 + p
    S_e = np.concatenate([seg[:, 0, :], seg[:, 1, :]], axis=0)  # [256, 512] (d, c)
    S_z = np.concatenate([seg[:, 2, :], seg[:, 3, :]], axis=0)

    cntm = np.maximum(counts, 1.0)
    protos = (S_e / cntm).T  # [512, 256]

    # loss1: cross-entropy
    l1 = float(np.mean(np.log(ce_sums) - gls))

    # loss2: assignment loss
    l2 = float((sse - np.sum(counts * np.sum(protos * protos, axis=1))) / B)

    # loss3: SupCon
    lse = np.log(rowsums)
    v2 = counts >= 2.0
    t3b = float(lse[v2[labels]].sum())
    Sz2 = np.sum(S_z * S_z, axis=0)  # ||S_c||^2
    cm1 = np.maximum(counts - 1.0, 1.0)
    t3a = float(np.sum((Sz2 - counts)[v2] * INV_TAU / cm1[v2]))
    nvalid = float(counts[v2].sum())
    l3 = -(t3a - t3b) / max(nvalid, 1.0)

    # loss4: prototype orthogonality
    pnorm = np.sqrt(np.sum(protos * protos, axis=1))
    pn = protos / (pnorm + EPS)[:, None]
    present = counts > 0
    Psub = pn[present].astype(np.float64)
    G = Psub @ Psub.T
    npres = float(present.sum())
    l4 = float((np.sum(G * G) - np.sum(np.diag(G) ** 2))
               / max(npres * npres - npres, 1.0))

    total = l1 + ALPHA * l2 + BETA * l3 + GAMMA * l4
    return tuple(np.float32(v) for v in (total, l1, l2, l3, l4))


def kernel(logits, embeddings, labels):
    from concourse import bass_utils

    nc = _get_nc()
    logits = np.ascontiguousarray(np.asarray(logits, dtype=np.float32))
    embeddings = np.ascontiguousarray(np.asarray(embeddings, dtype=np.float32))
    labels_np = np.asarray(labels)

    in_maps = _pack_inputs(logits, embeddings, labels_np)
    res = bass_utils.run_bass_kernel_spmd(nc, in_maps, core_ids=list(range(NCORES)))
    return _finish(res.results, labels_np)
